# revision 9
# baseline (speedup 1.0000x reference)
"""Trainium2 Bass kernel for nn_FGE_18047452577900.

Math (per sample; verified in numpy against the jax reference):
  - The low/high frequency-mask split collapses algebraically:
        lr+hr = gelu(b1) + gelu(mix(xf)+b1) + xf      (everywhere)
    so the masks vanish.
  - softshrink(v) = v - clamp(v, -lam, +lam); folding the gelu(b1) constant
    into per-channel clamp bounds:  s = u - clamp(u, a_c, b_c)  with
    u = gelu(mix(xf)+b1) + xf,  a_c = -lam - gelu(b1)_c, b_c = lam - gelu(b1)_c.
  - irfft2(rfft2(x)) = x, so the "origin" residual is s_full = s + xf before
    the inverse transform; y = irfft2(s_full), then 1x1-conv projection and
    training-mode BatchNorm (global batch stats -> cross-core AllReduce) + gelu.
  - rfft2 / irfft2 are dense 64-point DFT matmuls on the PE, staged as:
        S1 (H-DFT, x as 2-channel stationary)  -> ZT [(c2,w), h'RI]
        S2 (W-DFT, ZT as stationary)           -> [h'RI, (w'C|w'S)]
        combine (DVE)                          -> pixel-major Xf
        PE-transpose per w'                    -> channel-major XC [c, (w',h')]
        block-diag mix matmul + ACT/DVE elementwise chain -> s_full
        PE-transpose back, inverse H-DFT, inverse W-DFT (hermitian-folded),
        PE-transpose to channel-major y, projection matmul, BN stats,
        AllReduce [96,2], finalize, gelu, DMA out.

Sharding: pure data parallelism over batch B=64 across 8 cores (8 samples
per core); all parameters replicated; BN batch stats all-reduced.
"""

import math
import numpy as np

import concourse.bass as bass
import concourse.mybir as mybir
import concourse.tile as tile
from concourse.bass_utils import run_bass_kernel_spmd
from concourse.masks import make_identity

# ---------------------------------------------------------------- problem dims
B_TOT, C, H, W = 64, 96, 64, 64
NCORES = 8
BL = B_TOT // NCORES          # samples per core
WF = 33                       # rfft width
NG, BLK = 4, 24
LAM = 0.01
BN_EPS = 1e-5
NPIX = H * W                  # 4096
FPIX = WF * H                 # 2112 (freq pixels per channel)
N_STAT = float(B_TOT * NPIX)  # BN normalizer

F32 = mybir.dt.float32
MM_DT = mybir.dt.float32      # matmul operand dtype (fp32 first; bf16 later)
MM_NP = np.float32

MIX_CHUNK = 352               # 6 chunks over 2112
PROJ_CHUNK = 512              # 8 chunks over 4096


def _gelu_exact(v):
    v = np.asarray(v, dtype=np.float64)
    return 0.5 * v * (1.0 + np.vectorize(math.erf)(v / math.sqrt(2.0)))


def _consts(w1, b1, proj_w, proj_b, bn_gamma, bn_beta):
    """Host-side constant prep: DFT matrices, block-diag mix weights,
    clamp bounds. All small (<=128x128)."""
    n = np.arange(64, dtype=np.float64)
    th = 2 * np.pi * np.outer(n, n) / 64.0
    NORM = 1.0 / 8.0
    FhC = np.cos(th) * NORM
    FhS = -np.sin(th) * NORM
    fhri = np.concatenate([FhC, FhS], axis=1)            # [64,128] S1 moving

    tw = 2 * np.pi * np.outer(n, n[:WF]) / 64.0
    FwC = np.cos(tw) * NORM
    FwS = np.sin(tw) * NORM
    fwcs = np.concatenate([FwC, FwS], axis=1)            # [64,66] S2 moving

    FhIC = np.cos(th) * NORM
    FhIS = np.sin(th) * NORM
    fhinv_r = np.concatenate([FhIC, FhIS], axis=1)       # [64,128] IH moving (R)
    fhinv_i = np.concatenate([-FhIS, FhIC], axis=1)      # [64,128] IH moving (I)

    g = np.ones(WF)
    g[1:32] = 2.0
    CwI = (np.cos(tw.T) * NORM) * g[:, None]             # [33,64]
    SwI = (np.sin(tw.T) * NORM) * g[:, None]
    # IW stationaries, rows interleaved (w'*2 + cl), cols (cl*64 + w)
    cwid = np.zeros((66, 128))
    swidn = np.zeros((66, 128))
    for cl in range(2):
        cwid[cl::2, cl * 64:(cl + 1) * 64] = CwI
        swidn[cl::2, cl * 64:(cl + 1) * 64] = -SwI

    bd0 = np.zeros((C, C))
    bd1 = np.zeros((C, C))
    for k in range(NG):
        bd0[k * BLK:(k + 1) * BLK, k * BLK:(k + 1) * BLK] = w1[0][k]
        bd1[k * BLK:(k + 1) * BLK, k * BLK:(k + 1) * BLK] = w1[1][k]

    b1r = np.asarray(b1[0], np.float64).reshape(C)
    b1i = np.asarray(b1[1], np.float64).reshape(C)
    cr = _gelu_exact(b1r)
    ci = _gelu_exact(b1i)

    f32 = lambda a: np.ascontiguousarray(a, dtype=np.float32)
    mm = lambda a: np.ascontiguousarray(a, dtype=MM_NP)
    return {
        "c_fhri": mm(fhri),
        "c_fwcs": mm(np.concatenate([fwcs, fwcs], axis=0)),
        "c_fhinv_r": mm(fhinv_r),
        "c_fhinv_i": mm(fhinv_i),
        "c_cwid": mm(cwid),
        "c_swidn": mm(swidn),
        "c_bd0": mm(bd0),
        "c_bd1": mm(bd1),
        "c_projT": mm(np.asarray(proj_w, np.float64).T),
        "c_b1r": f32(b1r[:, None]),
        "c_b1i": f32(b1i[:, None]),
        "c_ar": f32((-LAM - cr)[:, None]),
        "c_br": f32((LAM - cr)[:, None]),
        "c_ai": f32((-LAM - ci)[:, None]),
        "c_bi": f32((LAM - ci)[:, None]),
        "c_projb": f32(np.asarray(proj_b, np.float64)[:, None]),
        "c_gamma": f32(np.asarray(bn_gamma, np.float64)[:, None]),
        "c_beta": f32(np.asarray(bn_beta, np.float64)[:, None]),
    }


# ------------------------------------------------------- walrus wait-limit fix
_carrier_counter = [0]


def _split_sync_waits(nc, limit=1):
    """This container's walrus codegen rejects instructions carrying more
    than one sync-wait. Move excess waits onto EventSemaphore carrier
    instructions inserted immediately before, on the same engine."""
    si_type = None
    n_split = 0
    for f in nc.m.functions:
        for blk in f.blocks:
            il = list(blk.instructions)
            out = []
            changed = False
            for ins in il:
                si = ins.sync_info
                if si is not None and len(si.on_wait) > limit:
                    if si_type is None:
                        si_type = type(si)
                    waits = list(si.on_wait)
                    for w in waits[:-limit]:
                        _carrier_counter[0] += 1
                        c = mybir.InstEventSemaphore(
                            name=f"waitcarrier_{_carrier_counter[0]}",
                            ins=[], outs=[],
                        )
                        c.engine = ins.engine
                        c.sync_info = si_type(on_wait=[w], on_update=[])
                        out.append(c)
                    si.on_wait = waits[-limit:]
                    changed = True
                    n_split += 1
                out.append(ins)
            if changed:
                blk.instructions = out
    return n_split


# ------------------------------------------------------------- kernel builder
def _build_nc():
    nc = bass.Bass(num_devices=NCORES)
    AP = bass.AP

    x_d = nc.dram_tensor("x", [BL, C, H, W], F32, kind="ExternalInput")
    out_d = nc.dram_tensor("out", [BL, C, H, W], F32, kind="ExternalOutput")
    y2_d = nc.dram_tensor("y2buf", [BL, C, NPIX], F32)
    cc_in = nc.dram_tensor("cc_in", [C, 2], F32)
    cc_out = nc.dram_tensor("cc_out", [C, 2], F32, addr_space="Shared")

    cn = {}  # constant dram tensors
    for name, shape, dt in [
        ("c_fhri", [64, 128], MM_DT), ("c_fwcs", [128, 66], MM_DT),
        ("c_fhinv_r", [64, 128], MM_DT), ("c_fhinv_i", [64, 128], MM_DT),
        ("c_cwid", [66, 128], MM_DT), ("c_swidn", [66, 128], MM_DT),
        ("c_bd0", [C, C], MM_DT), ("c_bd1", [C, C], MM_DT),
        ("c_projT", [C, C], MM_DT),
        ("c_b1r", [C, 1], F32), ("c_b1i", [C, 1], F32),
        ("c_ar", [C, 1], F32), ("c_br", [C, 1], F32),
        ("c_ai", [C, 1], F32), ("c_bi", [C, 1], F32),
        ("c_projb", [C, 1], F32), ("c_gamma", [C, 1], F32),
        ("c_beta", [C, 1], F32),
    ]:
        cn[name] = nc.dram_tensor(name, shape, dt, kind="ExternalInput")

    from contextlib import ExitStack

    with tile.TileContext(nc) as tc, ExitStack() as stack:
        con = stack.enter_context(tc.tile_pool(name="con", bufs=1))
        big = stack.enter_context(tc.tile_pool(name="big", bufs=1))
        xin = stack.enter_context(tc.tile_pool(name="xin", bufs=1))
        sm = stack.enter_context(tc.tile_pool(name="sm", bufs=2))
        ps = stack.enter_context(tc.tile_pool(name="ps", bufs=7, space="PSUM"))

        # ---- constants into SBUF
        ct = {}
        for name, t in cn.items():
            shape = list(t.shape)
            dt = MM_DT if name in (
                "c_fhri", "c_fwcs", "c_fhinv_r", "c_fhinv_i",
                "c_cwid", "c_swidn", "c_bd0", "c_bd1", "c_projT",
            ) else F32
            st = con.tile(shape, dt, tag=name)
            nc.sync.dma_start(st[:, :], t[:, :])
            ct[name] = st
        ident = con.tile([64, 64], F32, tag="ident")
        make_identity(nc, ident[:, :])
        ident96 = con.tile([96, 96], F32, tag="ident96")
        make_identity(nc, ident96[:, :])

        # ---- per-sample working tensors (reused across samples)
        # pixel-major freq cubes [h', w', c]
        xpr = big.tile([64, WF, C], F32, tag="xpr")
        xpi = big.tile([64, WF, C], F32, tag="xpi")
        # channel-major freq cubes [c, w', h']
        xcr = big.tile([C, WF, 64], F32, tag="xcr")
        xci = big.tile([C, WF, 64], F32, tag="xci")
        scr = big.tile([C, WF, 64], F32, tag="scr")
        sci = big.tile([C, WF, 64], F32, tag="sci")
        # pixel-major s (post-T2) [h', w', c]
        strr = big.tile([64, C // 2, WF, 2], MM_DT, tag="strr")
        stri = big.tile([64, C // 2, WF, 2], MM_DT, tag="stri")
        # inverse intermediates
        ypw = big.tile([64, H, C], F32, tag="ypw")       # [w, h, c]
        yc = big.tile([C, NPIX], F32, tag="yc")          # [c, (h,w)]
        # BN partial sums
        sums = big.tile([C, BL * 8], F32, tag="sums")
        sqs = big.tile([C, BL * 8], F32, tag="sqs")

        for b in range(BL):
            # ---- A: load x[b] h-major: [h, c, w]
            xt = xin.tile([64, C, W], F32, tag="xt")
            nc.sync.dma_start(
                xt[:, :, :],
                AP(x_d, b * C * NPIX, [[W, 64], [NPIX, C], [1, W]]),
            )

            # ---- S1 (H-DFT) + evict, S2 (W-DFT) + combine
            for cp in range(C // 2):
                zt_ps = ps.tile([128, 128], F32, tag="ps")
                nc.tensor.matmul(
                    zt_ps[:, :], xt[:, 2 * cp:2 * cp + 2, :].opt(), ct["c_fhri"][:, :],
                    start=True, stop=True,
                )
                zt = sm.tile([128, 128], MM_DT, tag="zt")
                if cp % 2 == 0:
                    nc.vector.tensor_copy(zt[:, :], zt_ps[:, :])
                else:
                    nc.scalar.copy(zt[:, :], zt_ps[:, :])
                for cl in range(2):
                    c = 2 * cp + cl
                    zrow = zt[64 * cl:64 * (cl + 1), :]
                    frow = ct["c_fwcs"][64 * cl:64 * (cl + 1), :]
                    psA = ps.tile([128, WF], F32, tag="ps")
                    nc.tensor.matmul(
                        psA[:, :], zrow, frow[:, 0:WF], start=True, stop=True)
                    psB = ps.tile([128, WF], F32, tag="ps")
                    nc.tensor.matmul(
                        psB[:, :], zrow, frow[:, WF:66], start=True, stop=True)
                    sbB = sm.tile([128, WF], F32, tag="sbB")
                    nc.scalar.copy(sbB[:, :], psB[:, :])
                    # XfR = P_RC + P_IS ; XfI = P_IC - P_RS  -> [h',(w',c)]
                    nc.vector.tensor_add(
                        xpr[:, :, c], psA[0:64, :], sbB[64:128, :])
                    nc.vector.tensor_sub(
                        xpi[:, :, c], psA[64:128, :], sbB[0:64, :])

            # ---- T1: pixel-major -> channel-major (per w')
            for wq in range(WF):
                for src, dst in ((xpr, xcr), (xpi, xci)):
                    t_ps = ps.tile([C, 64], F32, tag="ps")
                    nc.tensor.transpose(t_ps[:, :], src[:, wq, :], ident[:, :])
                    if wq % 2 == 0:
                        nc.vector.tensor_copy(dst[:, wq, :], t_ps[:, :])
                    else:
                        nc.scalar.copy(dst[:, wq, :], t_ps[:, :])

            # ---- MIX + elementwise chain (channel-major), 6 chunks of 352
            for (xc_t, s_t, bd, b1t, at, bt) in (
                (xcr, scr, "c_bd0", "c_b1r", "c_ar", "c_br"),
                (xci, sci, "c_bd1", "c_b1i", "c_ai", "c_bi"),
            ):
                xc_f = xc_t[:, :, :].rearrange("p a b -> p (a b)")
                s_f = s_t[:, :, :].rearrange("p a b -> p (a b)")
                for ch in range(FPIX // MIX_CHUNK):
                    sl = slice(ch * MIX_CHUNK, (ch + 1) * MIX_CHUNK)
                    m_ps = ps.tile([C, MIX_CHUNK], F32, tag="ps")
                    nc.tensor.matmul(
                        m_ps[:, :], ct[bd][:, :], xc_f[:, sl],
                        start=True, stop=True,
                    )
                    t_sb = sm.tile([C, MIX_CHUNK], F32, tag="t_sb")
                    nc.scalar.activation(
                        t_sb[:, :], m_ps[:, :],
                        mybir.ActivationFunctionType.Gelu,
                        bias=ct[b1t][:, 0:1], scale=1.0,
                    )
                    u_sb = sm.tile([C, MIX_CHUNK], F32, tag="u_sb")
                    nc.vector.tensor_add(u_sb[:, :], t_sb[:, :], xc_f[:, sl])
                    cl_sb = sm.tile([C, MIX_CHUNK], F32, tag="cl_sb")
                    nc.vector.tensor_scalar(
                        cl_sb[:, :], u_sb[:, :],
                        ct[at][:, 0:1], ct[bt][:, 0:1],
                        op0=mybir.AluOpType.max, op1=mybir.AluOpType.min,
                    )
                    d_sb = sm.tile([C, MIX_CHUNK], F32, tag="d_sb")
                    nc.vector.scalar_tensor_tensor(
                        d_sb[:, :], cl_sb[:, :], -1.0, u_sb[:, :],
                        op0=mybir.AluOpType.mult, op1=mybir.AluOpType.add,
                    )
                    nc.vector.tensor_add(s_f[:, sl], d_sb[:, :], xc_f[:, sl])

            # ---- T2: channel-major s -> pixel-major (per w')
            for wq in range(WF):
                for src, dst in ((scr, strr), (sci, stri)):
                    t_ps = ps.tile([64, C], F32, tag="ps")
                    nc.tensor.transpose(t_ps[:, :], src[:, wq, :], ident96[:, :])
                    if wq % 2 == 0:
                        nc.vector.tensor_copy(dst[:, :, wq, :], t_ps[:, :])
                    else:
                        nc.scalar.copy(dst[:, :, wq, :], t_ps[:, :])

            # ---- IH (inverse H-DFT) + IW (inverse W-DFT, hermitian fold)
            for cp in range(C // 2):
                ih_ps = ps.tile([66, 128], F32, tag="ps")
                nc.tensor.matmul(
                    ih_ps[:, :], strr[:, cp, :, :].opt(),
                    ct["c_fhinv_r"][:, :], start=True, stop=False,
                )
                nc.tensor.matmul(
                    ih_ps[:, :], stri[:, cp, :, :].opt(),
                    ct["c_fhinv_i"][:, :], start=False, stop=True,
                )
                zinv = sm.tile([66, 128], MM_DT, tag="zinv")
                if cp % 2 == 0:
                    nc.vector.tensor_copy(zinv[:, :], ih_ps[:, :])
                else:
                    nc.scalar.copy(zinv[:, :], ih_ps[:, :])
                yw_ps = ps.tile([128, 64], F32, tag="ps")
                nc.tensor.matmul(
                    yw_ps[:, :], ct["c_cwid"][:, :], zinv[:, 0:64],
                    start=True, stop=False,
                )
                nc.tensor.matmul(
                    yw_ps[:, :], ct["c_swidn"][:, :], zinv[:, 64:128],
                    start=False, stop=True,
                )
                # rows (cl*64 + w) x cols h -> ypw [w, h, c]
                for cl in range(2):
                    c = 2 * cp + cl
                    if c % 2 == 0:
                        nc.vector.tensor_copy(
                            ypw[:, :, c], yw_ps[64 * cl:64 * (cl + 1), :])
                    else:
                        nc.scalar.copy(
                            ypw[:, :, c], yw_ps[64 * cl:64 * (cl + 1), :])

            # ---- T3: [w, h, c] -> channel-major y [c, (h,w)]
            yc3 = yc[:, :].rearrange("p (a b) -> p a b", a=H)
            for h in range(H):
                t_ps = ps.tile([C, 64], F32, tag="ps")
                nc.tensor.transpose(t_ps[:, :], ypw[:, h, :], ident[:, :])
                if h % 2 == 0:
                    nc.vector.tensor_copy(yc3[:, h, :], t_ps[:, :])
                else:
                    nc.scalar.copy(yc3[:, h, :], t_ps[:, :])

            # ---- projection + BN partial sums; y2 spilled to DRAM
            for ch in range(NPIX // PROJ_CHUNK):
                sl = slice(ch * PROJ_CHUNK, (ch + 1) * PROJ_CHUNK)
                p_ps = ps.tile([C, PROJ_CHUNK], F32, tag="ps")
                nc.tensor.matmul(
                    p_ps[:, :], ct["c_projT"][:, :], yc[:, sl],
                    start=True, stop=True,
                )
                col = b * 8 + ch
                y2_sb = sm.tile([C, PROJ_CHUNK], F32, tag="y2_sb")
                nc.scalar.activation(
                    y2_sb[:, :], p_ps[:, :],
                    mybir.ActivationFunctionType.Identity,
                    bias=ct["c_projb"][:, 0:1], scale=1.0,
                    accum_out=sums[:, col:col + 1],
                )
                sq_sb = sm.tile([C, PROJ_CHUNK], F32, tag="sq_sb")
                nc.scalar.activation(
                    sq_sb[:, :], y2_sb[:, :],
                    mybir.ActivationFunctionType.Square,
                    accum_out=sqs[:, col:col + 1],
                )
                nc.sync.dma_start(
                    AP(y2_d, b * C * NPIX + ch * PROJ_CHUNK,
                       [[NPIX, C], [1, PROJ_CHUNK]]),
                    y2_sb[:, :],
                )

        # ---- BN stats: local reduce, AllReduce, finalize
        stat = big.tile([C, 2], F32, tag="stat")
        nc.vector.tensor_reduce(
            stat[:, 0:1], sums[:, :], axis=mybir.AxisListType.X,
            op=mybir.AluOpType.add)
        nc.vector.tensor_reduce(
            stat[:, 1:2], sqs[:, :], axis=mybir.AxisListType.X,
            op=mybir.AluOpType.add)
        nc.sync.dma_start(cc_in[:, :], stat[:, :])
        nc.gpsimd.collective_compute(
            "AllReduce", mybir.AluOpType.add,
            replica_groups=[list(range(NCORES))],
            ins=[cc_in.ap().opt()], outs=[cc_out.ap().opt()],
        )
        gstat = big.tile([C, 2], F32, tag="gstat")
        nc.sync.dma_start(gstat[:, :], cc_out[:, :])

        mean = big.tile([C, 1], F32, tag="mean")
        nc.vector.tensor_scalar(
            mean[:, :], gstat[:, 0:1], 1.0 / N_STAT, None,
            op0=mybir.AluOpType.mult)
        e2 = big.tile([C, 1], F32, tag="e2")
        nc.vector.tensor_scalar(
            e2[:, :], gstat[:, 1:2], 1.0 / N_STAT, None,
            op0=mybir.AluOpType.mult)
        msq = big.tile([C, 1], F32, tag="msq")
        nc.vector.tensor_mul(msq[:, :], mean[:, :], mean[:, :])
        var = big.tile([C, 1], F32, tag="var")
        nc.vector.tensor_sub(var[:, :], e2[:, :], msq[:, :])
        veps = big.tile([C, 1], F32, tag="veps")
        nc.vector.tensor_scalar(
            veps[:, :], var[:, :], BN_EPS, None, op0=mybir.AluOpType.add)
        std = big.tile([C, 1], F32, tag="std")
        nc.scalar.activation(
            std[:, :], veps[:, :], mybir.ActivationFunctionType.Sqrt)
        rstd = big.tile([C, 1], F32, tag="rstd")
        nc.vector.reciprocal(rstd[:, :], std[:, :])
        bscale = big.tile([C, 1], F32, tag="bscale")
        nc.vector.tensor_mul(bscale[:, :], ct["c_gamma"][:, 0:1], rstd[:, :])
        mscale = big.tile([C, 1], F32, tag="mscale")
        nc.vector.tensor_mul(mscale[:, :], mean[:, :], bscale[:, :])
        bshift = big.tile([C, 1], F32, tag="bshift")
        nc.vector.tensor_sub(bshift[:, :], ct["c_beta"][:, 0:1], mscale[:, :])

        # ---- final pass: gelu(scale*y2 + shift) -> out
        for b in range(BL):
            for ch in range(NPIX // PROJ_CHUNK):
                y2t = sm.tile([C, PROJ_CHUNK], F32, tag="y2t")
                nc.sync.dma_start(
                    y2t[:, :],
                    AP(y2_d, b * C * NPIX + ch * PROJ_CHUNK,
                       [[NPIX, C], [1, PROJ_CHUNK]]),
                )
                ot = sm.tile([C, PROJ_CHUNK], F32, tag="ot")
                nc.scalar.activation(
                    ot[:, :], y2t[:, :], mybir.ActivationFunctionType.Gelu,
                    bias=bshift[:, 0:1], scale=bscale[:, 0:1],
                )
                nc.sync.dma_start(
                    AP(out_d, b * C * NPIX + ch * PROJ_CHUNK,
                       [[NPIX, C], [1, PROJ_CHUNK]]),
                    ot[:, :],
                )

    _split_sync_waits(nc)
    return nc


_NC_CACHE = None


def _get_nc():
    global _NC_CACHE
    if _NC_CACHE is None:
        _NC_CACHE = _build_nc()
    return _NC_CACHE


def kernel(x, w1, b1, proj_w, proj_b, bn_gamma, bn_beta):
    x = np.ascontiguousarray(np.asarray(x), dtype=np.float32)
    consts = _consts(w1, b1, proj_w, proj_b, bn_gamma, bn_beta)
    nc = _get_nc()
    in_maps = []
    for i in range(NCORES):
        m = {"x": np.ascontiguousarray(x[i * BL:(i + 1) * BL])}
        m.update(consts)
        in_maps.append(m)
    res = run_bass_kernel_spmd(nc, in_maps, core_ids=list(range(NCORES)))
    return np.concatenate([res.results[i]["out"] for i in range(NCORES)], axis=0)


# revision 10
# speedup vs baseline: 1.6588x; 1.6588x over previous
"""Trainium2 Bass kernel for nn_FGE_18047452577900.

Math (per sample; verified in numpy against the jax reference):
  - The low/high frequency-mask split collapses algebraically:
        lr+hr = gelu(b1) + gelu(mix(xf)+b1) + xf      (everywhere)
    so the masks vanish.
  - softshrink(v) = v - clamp(v, -lam, +lam); folding the gelu(b1) constant
    into per-channel clamp bounds:  s = u - clamp(u, a_c, b_c)  with
    u = gelu(mix(xf)+b1) + xf,  a_c = -lam - gelu(b1)_c, b_c = lam - gelu(b1)_c.
  - irfft2(rfft2(x)) = x, so the "origin" residual is s_full = s + xf before
    the inverse transform; y = irfft2(s_full), then 1x1-conv projection and
    training-mode BatchNorm (global batch stats -> cross-core AllReduce) + gelu.
  - rfft2 / irfft2 are dense 64-point DFT matmuls on the PE, staged as:
        S1 (H-DFT, x as 2-channel stationary)  -> ZT [(c2,w), h'RI]
        S2 (W-DFT, ZT as stationary)           -> [h'RI, (w'C|w'S)]
        combine (DVE)                          -> pixel-major Xf
        PE-transpose per w'                    -> channel-major XC [c, (w',h')]
        block-diag mix matmul + ACT/DVE elementwise chain -> s_full
        PE-transpose back, inverse H-DFT, inverse W-DFT (hermitian-folded),
        PE-transpose to channel-major y, projection matmul, BN stats,
        AllReduce [96,2], finalize, gelu, DMA out.

Sharding: pure data parallelism over batch B=64 across 8 cores (8 samples
per core); all parameters replicated; BN batch stats all-reduced.
"""

import math
import numpy as np

import concourse.bass as bass
import concourse.mybir as mybir
import concourse.tile as tile
from concourse.bass_utils import run_bass_kernel_spmd
from concourse.masks import make_identity

# ---------------------------------------------------------------- problem dims
B_TOT, C, H, W = 64, 96, 64, 64
NCORES = 8
BL = B_TOT // NCORES          # samples per core
WF = 33                       # rfft width
NG, BLK = 4, 24
LAM = 0.01
BN_EPS = 1e-5
NPIX = H * W                  # 4096
FPIX = WF * H                 # 2112 (freq pixels per channel)
N_STAT = float(B_TOT * NPIX)  # BN normalizer

import ml_dtypes

F32 = mybir.dt.float32
FR = mybir.dt.float32r        # full-rate fp32 (rounded) for N>=256 matmuls
MM_DT = mybir.dt.bfloat16     # FFT-core matmul operand dtype
MM_NP = ml_dtypes.bfloat16

MIX_CHUNK = 352               # 6 chunks over 2112
PROJ_CHUNK = 512              # 8 chunks over 4096


def _gelu_exact(v):
    v = np.asarray(v, dtype=np.float64)
    return 0.5 * v * (1.0 + np.vectorize(math.erf)(v / math.sqrt(2.0)))


def _consts(w1, b1, proj_w, proj_b, bn_gamma, bn_beta):
    """Host-side constant prep: DFT matrices, block-diag mix weights,
    clamp bounds. All small (<=128x128)."""
    n = np.arange(64, dtype=np.float64)
    th = 2 * np.pi * np.outer(n, n) / 64.0
    NORM = 1.0 / 8.0
    FhC = np.cos(th) * NORM
    FhS = -np.sin(th) * NORM
    fhri = np.concatenate([FhC, FhS], axis=1)            # [64,128] S1 moving

    tw = 2 * np.pi * np.outer(n, n[:WF]) / 64.0
    FwC = np.cos(tw) * NORM
    FwS = np.sin(tw) * NORM
    fwcs = np.concatenate([FwC, FwS], axis=1)            # [64,66] S2 moving

    FhIC = np.cos(th) * NORM
    FhIS = np.sin(th) * NORM
    fhinv_r = np.concatenate([FhIC, FhIS], axis=1)       # [64,128] IH moving (R)
    fhinv_i = np.concatenate([-FhIS, FhIC], axis=1)      # [64,128] IH moving (I)

    g = np.ones(WF)
    g[1:32] = 2.0
    CwI = (np.cos(tw.T) * NORM) * g[:, None]             # [33,64]
    SwI = (np.sin(tw.T) * NORM) * g[:, None]
    # IW stationaries, rows interleaved (w'*2 + cl), cols (cl*64 + w)
    cwid = np.zeros((66, 128))
    swidn = np.zeros((66, 128))
    for cl in range(2):
        cwid[cl::2, cl * 64:(cl + 1) * 64] = CwI
        swidn[cl::2, cl * 64:(cl + 1) * 64] = -SwI

    bd0 = np.zeros((C, C))
    bd1 = np.zeros((C, C))
    for k in range(NG):
        bd0[k * BLK:(k + 1) * BLK, k * BLK:(k + 1) * BLK] = w1[0][k]
        bd1[k * BLK:(k + 1) * BLK, k * BLK:(k + 1) * BLK] = w1[1][k]

    b1r = np.asarray(b1[0], np.float64).reshape(C)
    b1i = np.asarray(b1[1], np.float64).reshape(C)
    cr = _gelu_exact(b1r)
    ci = _gelu_exact(b1i)

    f32 = lambda a: np.ascontiguousarray(a, dtype=np.float32)
    mm = lambda a: np.ascontiguousarray(a, dtype=MM_NP)
    return {
        "c_fhri": mm(fhri),
        "c_fwcs": mm(np.concatenate([fwcs, fwcs], axis=0)),
        "c_fhinv_r": mm(fhinv_r),
        "c_fhinv_i": mm(fhinv_i),
        "c_cwid": mm(cwid),
        "c_swidn": mm(swidn),
        "c_bd0": f32(bd0),
        "c_bd1": f32(bd1),
        "c_projT": f32(np.asarray(proj_w, np.float64).T),
        "c_b1r": f32(b1r[:, None]),
        "c_b1i": f32(b1i[:, None]),
        "c_ar": f32((-LAM - cr)[:, None]),
        "c_br": f32((LAM - cr)[:, None]),
        "c_ai": f32((-LAM - ci)[:, None]),
        "c_bi": f32((LAM - ci)[:, None]),
        "c_projb": f32(np.asarray(proj_b, np.float64)[:, None]),
        "c_gamma": f32(np.asarray(bn_gamma, np.float64)[:, None]),
        "c_beta": f32(np.asarray(bn_beta, np.float64)[:, None]),
    }


# ------------------------------------------------------- walrus wait-limit fix
_carrier_counter = [0]


def _split_sync_waits(nc, limit=1):
    """This container's walrus codegen rejects instructions carrying more
    than one sync-wait. Move excess waits onto EventSemaphore carrier
    instructions inserted immediately before, on the same engine."""
    si_type = None
    n_split = 0
    for f in nc.m.functions:
        for blk in f.blocks:
            il = list(blk.instructions)
            out = []
            changed = False
            for ins in il:
                si = ins.sync_info
                if si is not None and len(si.on_wait) > limit:
                    if si_type is None:
                        si_type = type(si)
                    waits = list(si.on_wait)
                    for w in waits[:-limit]:
                        _carrier_counter[0] += 1
                        c = mybir.InstEventSemaphore(
                            name=f"waitcarrier_{_carrier_counter[0]}",
                            ins=[], outs=[],
                        )
                        c.engine = ins.engine
                        c.sync_info = si_type(on_wait=[w], on_update=[])
                        out.append(c)
                    si.on_wait = waits[-limit:]
                    changed = True
                    n_split += 1
                out.append(ins)
            if changed:
                blk.instructions = out
    return n_split


# ------------------------------------------------------------- kernel builder
def _build_nc():
    nc = bass.Bass(num_devices=NCORES)
    AP = bass.AP

    x_d = nc.dram_tensor("x", [BL, C, H, W], F32, kind="ExternalInput")
    out_d = nc.dram_tensor("out", [BL, C, H, W], F32, kind="ExternalOutput")
    y2_d = nc.dram_tensor("y2buf", [BL, C, NPIX], F32)
    cc_in = nc.dram_tensor("cc_in", [C, 2], F32)
    cc_out = nc.dram_tensor("cc_out", [C, 2], F32, addr_space="Shared")

    cn = {}  # constant dram tensors
    for name, shape, dt in [
        ("c_fhri", [64, 128], MM_DT), ("c_fwcs", [128, 66], MM_DT),
        ("c_fhinv_r", [64, 128], MM_DT), ("c_fhinv_i", [64, 128], MM_DT),
        ("c_cwid", [66, 128], MM_DT), ("c_swidn", [66, 128], MM_DT),
        ("c_bd0", [C, C], FR), ("c_bd1", [C, C], FR),
        ("c_projT", [C, C], FR),
        ("c_b1r", [C, 1], F32), ("c_b1i", [C, 1], F32),
        ("c_ar", [C, 1], F32), ("c_br", [C, 1], F32),
        ("c_ai", [C, 1], F32), ("c_bi", [C, 1], F32),
        ("c_projb", [C, 1], F32), ("c_gamma", [C, 1], F32),
        ("c_beta", [C, 1], F32),
    ]:
        cn[name] = nc.dram_tensor(name, shape, dt, kind="ExternalInput")

    from contextlib import ExitStack

    with tile.TileContext(nc) as tc, ExitStack() as stack:
        con = stack.enter_context(tc.tile_pool(name="con", bufs=1))
        big = stack.enter_context(tc.tile_pool(name="big", bufs=1))
        xin = stack.enter_context(tc.tile_pool(name="xin", bufs=1))
        sm = stack.enter_context(tc.tile_pool(name="sm", bufs=2))
        ps = stack.enter_context(tc.tile_pool(name="ps", bufs=7, space="PSUM"))

        # ---- constants into SBUF
        ct = {}
        for name, t in cn.items():
            shape = list(t.shape)
            if name in ("c_fhri", "c_fwcs", "c_fhinv_r", "c_fhinv_i",
                        "c_cwid", "c_swidn"):
                dt = MM_DT
            elif name in ("c_bd0", "c_bd1", "c_projT"):
                dt = FR
            else:
                dt = F32
            st = con.tile(shape, dt, tag=name)
            nc.sync.dma_start(st[:, :], t[:, :])
            ct[name] = st
        ident = con.tile([64, 64], F32, tag="ident")
        make_identity(nc, ident[:, :])
        ident96 = con.tile([96, 96], F32, tag="ident96")
        make_identity(nc, ident96[:, :])

        # ---- per-sample working tensors (reused across samples)
        # pixel-major freq cubes [h', w', c]
        xpr = big.tile([64, WF, C], F32, tag="xpr")
        xpi = big.tile([64, WF, C], F32, tag="xpi")
        # channel-major freq cubes [c, w', h']
        xcr = big.tile([C, WF, 64], FR, tag="xcr")
        xci = big.tile([C, WF, 64], FR, tag="xci")
        scr = big.tile([C, WF, 64], F32, tag="scr")
        sci = big.tile([C, WF, 64], F32, tag="sci")
        # pixel-major s (post-T2) [h', w', c]
        strr = big.tile([64, C // 2, WF, 2], MM_DT, tag="strr")
        stri = big.tile([64, C // 2, WF, 2], MM_DT, tag="stri")
        # inverse intermediates
        ypw = big.tile([64, H, C], F32, tag="ypw")       # [w, h, c]
        yc = big.tile([C, NPIX], FR, tag="yc")           # [c, (h,w)]
        # BN partial sums
        sums = big.tile([C, BL * 8], F32, tag="sums")
        sqs = big.tile([C, BL * 8], F32, tag="sqs")

        for b in range(BL):
            # ---- A: load x[b] h-major: [h, c, w]
            xt = xin.tile([64, C, W], F32, tag="xt")
            nc.sync.dma_start(
                xt[:, :, :],
                AP(x_d, b * C * NPIX, [[W, 64], [NPIX, C], [1, W]]),
            )
            xt16 = xin.tile([64, C, W], MM_DT, tag="xt16")
            nc.vector.tensor_copy(xt16[:, 0:48, :], xt[:, 0:48, :])
            nc.scalar.copy(xt16[:, 48:C, :], xt[:, 48:C, :])

            # ---- S1 (H-DFT) + evict, S2 (W-DFT) + combine
            for cp in range(C // 2):
                zt_ps = ps.tile([128, 128], F32, tag="ps")
                nc.tensor.matmul(
                    zt_ps[:, :], xt16[:, 2 * cp:2 * cp + 2, :].opt(), ct["c_fhri"][:, :],
                    start=True, stop=True,
                )
                zt = sm.tile([128, 128], MM_DT, tag="zt")
                if cp % 2 == 0:
                    nc.vector.tensor_copy(zt[:, :], zt_ps[:, :])
                else:
                    nc.scalar.copy(zt[:, :], zt_ps[:, :])
                for cl in range(2):
                    c = 2 * cp + cl
                    zrow = zt[64 * cl:64 * (cl + 1), :]
                    frow = ct["c_fwcs"][64 * cl:64 * (cl + 1), :]
                    psA = ps.tile([128, WF], F32, tag="ps")
                    nc.tensor.matmul(
                        psA[:, :], zrow, frow[:, 0:WF], start=True, stop=True)
                    psB = ps.tile([128, WF], F32, tag="ps")
                    nc.tensor.matmul(
                        psB[:, :], zrow, frow[:, WF:66], start=True, stop=True)
                    sbB = sm.tile([128, WF], F32, tag="sbB")
                    nc.scalar.copy(sbB[:, :], psB[:, :])
                    # XfR = P_RC + P_IS ; XfI = P_IC - P_RS  -> [h',(w',c)]
                    nc.vector.tensor_add(
                        xpr[:, :, c], psA[0:64, :], sbB[64:128, :])
                    nc.vector.tensor_sub(
                        xpi[:, :, c], psA[64:128, :], sbB[0:64, :])

            # ---- T1: pixel-major -> channel-major (per w')
            for wq in range(WF):
                for src, dst in ((xpr, xcr), (xpi, xci)):
                    t_ps = ps.tile([C, 64], F32, tag="ps")
                    nc.tensor.transpose(t_ps[:, :], src[:, wq, :], ident[:, :])
                    if wq % 2 == 0:
                        nc.vector.tensor_copy(dst[:, wq, :], t_ps[:, :])
                    else:
                        nc.scalar.copy(dst[:, wq, :], t_ps[:, :])

            # ---- MIX + elementwise chain (channel-major), 6 chunks of 352
            for (xc_t, s_t, bd, b1t, at, bt) in (
                (xcr, scr, "c_bd0", "c_b1r", "c_ar", "c_br"),
                (xci, sci, "c_bd1", "c_b1i", "c_ai", "c_bi"),
            ):
                xc_fr = xc_t[:, :, :].rearrange("p a b -> p (a b)")
                xc_f = xc_fr.bitcast(F32)
                s_f = s_t[:, :, :].rearrange("p a b -> p (a b)")
                for ch in range(FPIX // MIX_CHUNK):
                    sl = slice(ch * MIX_CHUNK, (ch + 1) * MIX_CHUNK)
                    m_ps = ps.tile([C, MIX_CHUNK], F32, tag="ps")
                    nc.tensor.matmul(
                        m_ps[:, :], ct[bd][:, :], xc_fr[:, sl],
                        start=True, stop=True,
                    )
                    t_sb = sm.tile([C, MIX_CHUNK], F32, tag="t_sb")
                    nc.scalar.activation(
                        t_sb[:, :], m_ps[:, :],
                        mybir.ActivationFunctionType.Gelu,
                        bias=ct[b1t][:, 0:1], scale=1.0,
                    )
                    u_sb = sm.tile([C, MIX_CHUNK], F32, tag="u_sb")
                    nc.vector.tensor_add(u_sb[:, :], t_sb[:, :], xc_f[:, sl])
                    cl_sb = sm.tile([C, MIX_CHUNK], F32, tag="cl_sb")
                    nc.vector.tensor_scalar(
                        cl_sb[:, :], u_sb[:, :],
                        ct[at][:, 0:1], ct[bt][:, 0:1],
                        op0=mybir.AluOpType.max, op1=mybir.AluOpType.min,
                    )
                    d_sb = sm.tile([C, MIX_CHUNK], F32, tag="d_sb")
                    nc.vector.scalar_tensor_tensor(
                        d_sb[:, :], cl_sb[:, :], -1.0, u_sb[:, :],
                        op0=mybir.AluOpType.mult, op1=mybir.AluOpType.add,
                    )
                    nc.vector.tensor_add(s_f[:, sl], d_sb[:, :], xc_f[:, sl])

            # ---- T2: channel-major s -> pixel-major (per w')
            for wq in range(WF):
                for src, dst in ((scr, strr), (sci, stri)):
                    t_ps = ps.tile([64, C], F32, tag="ps")
                    nc.tensor.transpose(t_ps[:, :], src[:, wq, :], ident96[:, :])
                    if wq % 2 == 0:
                        nc.vector.tensor_copy(dst[:, :, wq, :], t_ps[:, :])
                    else:
                        nc.scalar.copy(dst[:, :, wq, :], t_ps[:, :])

            # ---- IH (inverse H-DFT) + IW (inverse W-DFT, hermitian fold)
            for cp in range(C // 2):
                ih_ps = ps.tile([66, 128], F32, tag="ps")
                nc.tensor.matmul(
                    ih_ps[:, :], strr[:, cp, :, :].opt(),
                    ct["c_fhinv_r"][:, :], start=True, stop=False,
                )
                nc.tensor.matmul(
                    ih_ps[:, :], stri[:, cp, :, :].opt(),
                    ct["c_fhinv_i"][:, :], start=False, stop=True,
                )
                zinv = sm.tile([66, 128], MM_DT, tag="zinv")
                if cp % 2 == 0:
                    nc.vector.tensor_copy(zinv[:, :], ih_ps[:, :])
                else:
                    nc.scalar.copy(zinv[:, :], ih_ps[:, :])
                yw_ps = ps.tile([128, 64], F32, tag="ps")
                nc.tensor.matmul(
                    yw_ps[:, :], ct["c_cwid"][:, :], zinv[:, 0:64],
                    start=True, stop=False,
                )
                nc.tensor.matmul(
                    yw_ps[:, :], ct["c_swidn"][:, :], zinv[:, 64:128],
                    start=False, stop=True,
                )
                # rows (cl*64 + w) x cols h -> ypw [w, h, c]
                for cl in range(2):
                    c = 2 * cp + cl
                    if c % 2 == 0:
                        nc.vector.tensor_copy(
                            ypw[:, :, c], yw_ps[64 * cl:64 * (cl + 1), :])
                    else:
                        nc.scalar.copy(
                            ypw[:, :, c], yw_ps[64 * cl:64 * (cl + 1), :])

            # ---- T3: [w, h, c] -> channel-major y [c, (h,w)]
            yc3 = yc[:, :].rearrange("p (a b) -> p a b", a=H)
            for h in range(H):
                t_ps = ps.tile([C, 64], F32, tag="ps")
                nc.tensor.transpose(t_ps[:, :], ypw[:, h, :], ident[:, :])
                if h % 2 == 0:
                    nc.vector.tensor_copy(yc3[:, h, :], t_ps[:, :])
                else:
                    nc.scalar.copy(yc3[:, h, :], t_ps[:, :])

            # ---- projection + BN partial sums; y2 spilled to DRAM
            for ch in range(NPIX // PROJ_CHUNK):
                sl = slice(ch * PROJ_CHUNK, (ch + 1) * PROJ_CHUNK)
                p_ps = ps.tile([C, PROJ_CHUNK], F32, tag="ps")
                nc.tensor.matmul(
                    p_ps[:, :], ct["c_projT"][:, :], yc[:, sl],
                    start=True, stop=True,
                )
                col = b * 8 + ch
                y2_sb = sm.tile([C, PROJ_CHUNK], F32, tag="y2_sb")
                nc.scalar.activation(
                    y2_sb[:, :], p_ps[:, :],
                    mybir.ActivationFunctionType.Identity,
                    bias=ct["c_projb"][:, 0:1], scale=1.0,
                    accum_out=sums[:, col:col + 1],
                )
                sq_sb = sm.tile([C, PROJ_CHUNK], F32, tag="sq_sb")
                nc.scalar.activation(
                    sq_sb[:, :], y2_sb[:, :],
                    mybir.ActivationFunctionType.Square,
                    accum_out=sqs[:, col:col + 1],
                )
                nc.sync.dma_start(
                    AP(y2_d, b * C * NPIX + ch * PROJ_CHUNK,
                       [[NPIX, C], [1, PROJ_CHUNK]]),
                    y2_sb[:, :],
                )

        # ---- BN stats: local reduce, AllReduce, finalize
        stat = big.tile([C, 2], F32, tag="stat")
        nc.vector.tensor_reduce(
            stat[:, 0:1], sums[:, :], axis=mybir.AxisListType.X,
            op=mybir.AluOpType.add)
        nc.vector.tensor_reduce(
            stat[:, 1:2], sqs[:, :], axis=mybir.AxisListType.X,
            op=mybir.AluOpType.add)
        nc.sync.dma_start(cc_in[:, :], stat[:, :])
        nc.gpsimd.collective_compute(
            "AllReduce", mybir.AluOpType.add,
            replica_groups=[list(range(NCORES))],
            ins=[cc_in.ap().opt()], outs=[cc_out.ap().opt()],
        )
        gstat = big.tile([C, 2], F32, tag="gstat")
        nc.sync.dma_start(gstat[:, :], cc_out[:, :])

        mean = big.tile([C, 1], F32, tag="mean")
        nc.vector.tensor_scalar(
            mean[:, :], gstat[:, 0:1], 1.0 / N_STAT, None,
            op0=mybir.AluOpType.mult)
        e2 = big.tile([C, 1], F32, tag="e2")
        nc.vector.tensor_scalar(
            e2[:, :], gstat[:, 1:2], 1.0 / N_STAT, None,
            op0=mybir.AluOpType.mult)
        msq = big.tile([C, 1], F32, tag="msq")
        nc.vector.tensor_mul(msq[:, :], mean[:, :], mean[:, :])
        var = big.tile([C, 1], F32, tag="var")
        nc.vector.tensor_sub(var[:, :], e2[:, :], msq[:, :])
        veps = big.tile([C, 1], F32, tag="veps")
        nc.vector.tensor_scalar(
            veps[:, :], var[:, :], BN_EPS, None, op0=mybir.AluOpType.add)
        std = big.tile([C, 1], F32, tag="std")
        nc.scalar.activation(
            std[:, :], veps[:, :], mybir.ActivationFunctionType.Sqrt)
        rstd = big.tile([C, 1], F32, tag="rstd")
        nc.vector.reciprocal(rstd[:, :], std[:, :])
        bscale = big.tile([C, 1], F32, tag="bscale")
        nc.vector.tensor_mul(bscale[:, :], ct["c_gamma"][:, 0:1], rstd[:, :])
        mscale = big.tile([C, 1], F32, tag="mscale")
        nc.vector.tensor_mul(mscale[:, :], mean[:, :], bscale[:, :])
        bshift = big.tile([C, 1], F32, tag="bshift")
        nc.vector.tensor_sub(bshift[:, :], ct["c_beta"][:, 0:1], mscale[:, :])

        # ---- final pass: gelu(scale*y2 + shift) -> out
        for b in range(BL):
            for ch in range(NPIX // PROJ_CHUNK):
                y2t = sm.tile([C, PROJ_CHUNK], F32, tag="y2t")
                nc.sync.dma_start(
                    y2t[:, :],
                    AP(y2_d, b * C * NPIX + ch * PROJ_CHUNK,
                       [[NPIX, C], [1, PROJ_CHUNK]]),
                )
                ot = sm.tile([C, PROJ_CHUNK], F32, tag="ot")
                nc.scalar.activation(
                    ot[:, :], y2t[:, :], mybir.ActivationFunctionType.Gelu,
                    bias=bshift[:, 0:1], scale=bscale[:, 0:1],
                )
                nc.sync.dma_start(
                    AP(out_d, b * C * NPIX + ch * PROJ_CHUNK,
                       [[NPIX, C], [1, PROJ_CHUNK]]),
                    ot[:, :],
                )

    _split_sync_waits(nc)
    return nc


_NC_CACHE = None


def _get_nc():
    global _NC_CACHE
    if _NC_CACHE is None:
        _NC_CACHE = _build_nc()
    return _NC_CACHE


def kernel(x, w1, b1, proj_w, proj_b, bn_gamma, bn_beta):
    x = np.ascontiguousarray(np.asarray(x), dtype=np.float32)
    consts = _consts(w1, b1, proj_w, proj_b, bn_gamma, bn_beta)
    nc = _get_nc()
    in_maps = []
    for i in range(NCORES):
        m = {"x": np.ascontiguousarray(x[i * BL:(i + 1) * BL])}
        m.update(consts)
        in_maps.append(m)
    res = run_bass_kernel_spmd(nc, in_maps, core_ids=list(range(NCORES)))
    return np.concatenate([res.results[i]["out"] for i in range(NCORES)], axis=0)
